# revision 30
# baseline (speedup 1.0000x reference)
"""AttnBlock3D Trainium2 Bass kernel (8 NeuronCores, SPMD).

Layout / algorithm (per core r, heads n = 2r, 2r+1):
  x viewed as [128=(t,c), 4096=hw].  BN stats computed on-device (sum / sumsq
  free-dim reduces + selection matmuls to combine over t per channel c).
  gamma/beta and all conv biases are folded on the host into block-diagonal
  projection weights so the device only normalizes with (x - mean) * rsqrt(var).
  All attention-path matmul operands are bf16 (fp32 matmul lowers to 2 HW
  passes at ~1us each; bf16 is ~8x faster).  PSUM accumulation stays fp32.
  q,k: one matmul each -> [64, hw] bf16 with head l at partitions l*32+f
  (32-aligned so the QK matmul's auto tile_position is legal).  v: per
  128-pixel chunk, lhsT = xhat chunk -> vT9 [hw, 9] per head (col 0 = ones;
  its matmul row accumulates sum(exp) for free).
  Attention per head: i-windows of [1536,1536,1024]; for each of 32 j-tiles:
  QK matmuls -> psum [128(j), width(i)], one big ACT Exp (scale=T^-0.5,
  no max subtraction -- scores are bounded, |s*scale| < 2.1) -> bf16 sbuf,
  then col-tiled AV matmuls accumulating [9, 512] per i-block at psum
  partitions 32g over all 32 j-tiles.  Unnormalized outputs + sumexp go
  through the AllGather; normalization happens once on the gathered tensor
  (one wide reciprocal instead of 16 single-partition ones).
  Output: gathered [144, hw] -> att_cf [(c,f), hw], recip broadcast via a
  DRAM bounce, one multiply, block-diag wp matmul, fused +bias +residual.
"""
import sys

import numpy as np

sys.path.insert(0, "/opt/trn_rl_repo")

T, C, HW, NCORES = 8, 16, 4096, 8
N_ELEM = T * HW  # per-channel element count for BN stats
EPS = 1e-5
SCALE = float(T) ** -0.5
# i-windows: (offset, width); widths chosen so qk psum = 3 banks, x2 buffers
IWIN = [(0, 1536), (1536, 1536), (3072, 1024)]

_CACHE = {}


def _build_program():
    import concourse.bass as bass
    import concourse.bacc as bacc
    import concourse.tile as tile
    from concourse import mybir

    f32 = mybir.dt.float32
    bf16 = mybir.dt.bfloat16
    AX = mybir.AxisListType
    OP = mybir.AluOpType
    ACT = mybir.ActivationFunctionType

    nc = bacc.Bacc("TRN2", target_bir_lowering=False, debug=False,
                   num_devices=NCORES)
    x = nc.dram_tensor("x", [128, HW], f32, kind="ExternalInput").ap()
    wq_bd = nc.dram_tensor("wq_bd", [128, 64], bf16, kind="ExternalInput").ap()
    wk_bd = nc.dram_tensor("wk_bd", [128, 64], bf16, kind="ExternalInput").ap()
    wv_rhs = nc.dram_tensor("wv_rhs", [128, 18], bf16,
                            kind="ExternalInput").ap()
    bq_col = nc.dram_tensor("bq_col", [64, 1], f32, kind="ExternalInput").ap()
    bk_col = nc.dram_tensor("bk_col", [64, 1], f32, kind="ExternalInput").ap()
    wp_bd = nc.dram_tensor("wp_bd", [128, 128], bf16,
                           kind="ExternalInput").ap()
    bp_col = nc.dram_tensor("bp_col", [128, 1], f32, kind="ExternalInput").ap()
    sel = nc.dram_tensor("sel", [128, 16], f32, kind="ExternalInput").ap()
    out = nc.dram_tensor("out", [128, HW], f32, kind="ExternalOutput").ap()

    with tile.TileContext(nc) as tc:
        with (
            tc.tile_pool(name="persist", bufs=1) as P1,
            tc.tile_pool(name="work", bufs=4) as PW,
            tc.tile_pool(name="scratch", bufs=1) as PS,
            tc.tile_pool(name="psq", bufs=2, space="PSUM") as PSQ,
            tc.tile_pool(name="psa", bufs=2, space="PSUM") as PSA,
            tc.tile_pool(name="dram", bufs=1, space="DRAM") as PD,
        ):
            # ---------------- loads ----------------
            x_sb = P1.tile([128, HW], f32)
            nc.sync.dma_start(out=x_sb, in_=x)
            wqbd_sb = P1.tile([128, 64], bf16)
            nc.sync.dma_start(out=wqbd_sb, in_=wq_bd)
            wkbd_sb = P1.tile([128, 64], bf16)
            nc.sync.dma_start(out=wkbd_sb, in_=wk_bd)
            wvrhs_sb = P1.tile([128, 18], bf16)
            nc.sync.dma_start(out=wvrhs_sb, in_=wv_rhs)
            bqcol_sb = P1.tile([64, 1], f32)
            nc.sync.dma_start(out=bqcol_sb, in_=bq_col)
            bkcol_sb = P1.tile([64, 1], f32)
            nc.sync.dma_start(out=bkcol_sb, in_=bk_col)
            wpbd_sb = P1.tile([128, 128], bf16)
            nc.sync.dma_start(out=wpbd_sb, in_=wp_bd)
            bpcol_sb = P1.tile([128, 1], f32)
            nc.sync.dma_start(out=bpcol_sb, in_=bp_col)
            sel_sb = P1.tile([128, 16], f32)
            nc.sync.dma_start(out=sel_sb, in_=sel)

            # ---------------- BN stats ----------------
            s1 = P1.tile([128, 2], f32)
            nc.vector.reduce_sum(out=s1[:, 0:1], in_=x_sb, axis=AX.X)
            xsq = PS.tile([128, HW], f32, tag="xsq")
            nc.vector.tensor_mul(xsq, x_sb, x_sb)
            nc.vector.reduce_sum(out=s1[:, 1:2], in_=xsq, axis=AX.X)
            ps_st = PSA.tile([1, 32], f32, tag="av")
            nc.tensor.matmul(ps_st[:, 0:16], lhsT=s1[:, 0:1], rhs=sel_sb,
                             start=True, stop=True)
            nc.tensor.matmul(ps_st[:, 16:32], lhsT=s1[:, 1:2], rhs=sel_sb,
                             start=True, stop=True)
            stats = P1.tile([1, 32], f32)
            nc.vector.tensor_scalar_mul(stats, ps_st, 1.0 / N_ELEM)
            var = P1.tile([1, 16], f32)
            nc.vector.tensor_mul(var, stats[:, 0:16], stats[:, 0:16])
            nc.vector.tensor_sub(var, stats[:, 16:32], var)
            eps_t = P1.tile([1, 1], f32)
            nc.vector.memset(eps_t, EPS)
            zero_t = P1.tile([1, 1], f32)
            nc.vector.memset(zero_t, 0.0)
            inv = P1.tile([1, 16], f32)
            nc.scalar.activation(inv, var, ACT.Ln, bias=eps_t)
            nc.scalar.activation(inv, inv, ACT.Exp, scale=-0.5, bias=zero_t)
            # bounce mean/inv through DRAM to broadcast [1,16] -> [128,1]
            st_dram = PD.tile([2, 16], f32)
            nc.sync.dma_start(out=st_dram[0:1, :], in_=stats[:, 0:16])
            nc.sync.dma_start(out=st_dram[1:2, :], in_=inv)
            mean_p = P1.tile([128, 1], f32)
            inv_p = P1.tile([128, 1], f32)
            for dst, row in ((mean_p, st_dram[0:1, :]),
                             (inv_p, st_dram[1:2, :])):
                src = bass.AP(tensor=row.tensor, offset=row.offset,
                              ap=[[0, T], list(row.ap[-1])])
                nc.gpsimd.dma_start(out=dst[:], in_=src)
            xhat = P1.tile([128, HW], bf16)
            nc.vector.tensor_scalar(out=xhat, in0=x_sb, scalar1=mean_p,
                                    scalar2=inv_p, op0=OP.subtract,
                                    op1=OP.mult)

            # ---------------- q/k projections (bf16) ----------------
            q_sb = P1.tile([64, HW], bf16)
            k_sb = P1.tile([64, HW], bf16)
            for dst, wbd, bcol in ((q_sb, wqbd_sb, bqcol_sb),
                                   (k_sb, wkbd_sb, bkcol_sb)):
                for ch in range(HW // 512):
                    ps = PSQ.tile([64, 512], f32, tag="qk")
                    nc.tensor.matmul(ps, lhsT=wbd,
                                     rhs=xhat[:, ch * 512:(ch + 1) * 512],
                                     start=True, stop=True)
                    nc.vector.tensor_scalar_add(
                        out=dst[:, ch * 512:(ch + 1) * 512], in0=ps,
                        scalar1=bcol)

            # ---------------- v -> vT9 per head (bf16, ones in col 0) ----
            vT9 = []
            for l in range(2):
                t9 = P1.tile([128, 32, 9], bf16, tag=f"t9_{l}")
                nc.vector.memset(t9[:, :, 0:1], 1.0)
                vT9.append(t9)
            for jc in range(32):
                psv = PSA.tile([128, 18], f32, tag="av")
                nc.tensor.matmul(psv, lhsT=xhat[:, jc * 128:(jc + 1) * 128],
                                 rhs=wvrhs_sb, start=True, stop=True)
                for l in range(2):
                    nc.vector.tensor_copy(vT9[l][:, jc, 1:9],
                                          psv[:, l * 9 + 1:l * 9 + 9])

            # ---------------- attention ----------------
            zero128 = P1.tile([128, 1], f32)
            nc.vector.memset(zero128, 0.0)
            cc_in = nc.dram_tensor("cc_in", [18, HW], f32).ap()
            # Heads interleaved per j-tile: doubles per-round PE work so the
            # PE stays busy past the ~3.4us HAM window (2.4 GHz instead of
            # 1.2), and the two heads' QK matmuls land in different row
            # groups (0 / 32) so they overlap on the array.
            for (i0, width) in IWIN:
                nblk = width // 512
                avs = []
                for l in range(2):
                    av_t = PSA.tile([128, 512], f32, tag="av",
                                    name=f"av_{l}_{i0}")
                    avs.append(av_t)
                # AV runs one j-tile behind QK/exp so the (in-order) PE
                # always has ready work while ACT computes the current exp.
                ex_prev = [None, None]
                for jt in range(33):
                    ex_cur = [None, None]
                    if jt < 32:
                        # block-interleaved issue: consecutive MMs hit row
                        # groups 0 / 32 alternately, so head pairs overlap
                        # on the array.
                        qks = []
                        for l in range(2):
                            qk_t = PSQ.tile([128, 1536], f32, tag="qk",
                                            name=f"qk_{l}")
                            qks.append(qk_t)
                        for b in range(nblk):
                            for l in range(2):
                                base = l * 32
                                nc.tensor.matmul(
                                    qks[l][:, b * 512:(b + 1) * 512],
                                    lhsT=k_sb[base:base + 8,
                                              jt * 128:(jt + 1) * 128],
                                    rhs=q_sb[base:base + 8,
                                             i0 + b * 512:i0 + (b + 1) * 512],
                                    start=True, stop=True)
                        for l in range(2):
                            ex = PW.tile([128, 1536], bf16, tag="ex")
                            nc.scalar.activation(ex[:, :width],
                                                 qks[l][:, :width],
                                                 ACT.Exp, scale=SCALE,
                                                 bias=zero128)
                            ex_cur[l] = ex
                    if jt > 0:
                        for l in range(2):
                            for g in range(nblk):
                                nc.tensor.matmul(
                                    avs[l][32 * g:32 * g + 9, :],
                                    lhsT=vT9[l][:, jt - 1, :],
                                    rhs=ex_prev[l][:, g * 512:(g + 1) * 512],
                                    start=(jt == 1), stop=(jt == 32),
                                    tile_position=(0, 32 * g),
                                    skip_group_check=True)
                    ex_prev = ex_cur
                # copy psum -> sbuf, ship unnormalized rows + sumexp
                for l in range(2):
                    s128 = PW.tile([128, 512], f32, tag="s128")
                    for g in range(nblk):
                        nc.vector.tensor_copy(s128[32 * g:32 * g + 9, :],
                                              avs[l][32 * g:32 * g + 9, :])
                        nc.sync.dma_start(
                            out=cc_in[l * 9:l * 9 + 9,
                                      i0 + g * 512:i0 + (g + 1) * 512],
                            in_=s128[32 * g:32 * g + 9, :])

            # ---------------- all-gather + normalize + output proj -------
            cc_out = nc.dram_tensor("cc_out", [NCORES * 18, HW], f32,
                                    addr_space="Shared").ap()
            nc.gpsimd.collective_compute(
                "AllGather", OP.bypass,
                replica_groups=[list(range(NCORES))],
                ins=[cc_in.opt()], outs=[cc_out.opt()])
            # reciprocal of all 16 heads' sumexp in one wide op:
            # rsum partition n*8+gc <- cc_out row n*9, cols gc*512..
            rsum = P1.tile([128, 512], f32)
            src = bass.AP(tensor=cc_out.tensor, offset=0,
                          ap=[[9 * HW, 16], [512, 8], [1, 512]])
            nc.sync.dma_start(out=rsum[:], in_=src)
            rinv = P1.tile([128, 512], f32)
            nc.vector.reciprocal(rinv, rsum)
            rdram = PD.tile([16, HW], f32)
            rd_t = rdram[:].tensor
            dst = bass.AP(tensor=rd_t, offset=0,
                          ap=[[HW, 16], [512, 8], [1, 512]])
            nc.sync.dma_start(out=dst, in_=rinv[:])
            # per-512-chunk pipeline: recip-bcast DMA || att DMA || mul ||
            # p-conv matmul || fused +bias+residual || out DMA
            for ch in range(HW // 512):
                c0 = ch * 512
                rbc = PW.tile([128, 512], f32, tag="rbc")
                src2 = bass.AP(tensor=rd_t, offset=c0,
                               ap=[[HW, 16], [0, T], [1, 512]])
                nc.sync.dma_start(out=rbc[:], in_=src2)
                acf = PW.tile([128, 512], f32, tag="acf")
                src3 = bass.AP(tensor=cc_out.tensor, offset=HW + c0,
                               ap=[[9 * HW, 16], [HW, T], [1, 512]])
                nc.sync.dma_start(out=acf[:], in_=src3)
                att_n = PW.tile([128, 512], bf16, tag="att_n")
                nc.vector.tensor_mul(att_n, acf, rbc)
                psp = PSQ.tile([128, 512], f32, tag="qk")
                nc.tensor.matmul(psp, lhsT=wpbd_sb, rhs=att_n,
                                 start=True, stop=True)
                och = PW.tile([128, 512], f32, tag="och")
                nc.vector.scalar_tensor_tensor(
                    out=och, in0=psp, scalar=bpcol_sb,
                    in1=x_sb[:, c0:c0 + 512], op0=OP.add, op1=OP.add)
                nc.sync.dma_start(out=out[:, c0:c0 + 512], in_=och)

    nc.compile()
    return nc


def host_inputs(r, x128, gamma, beta, wq, bq, wk, bk, wv, bv, wp, bp):
    """Per-core host-side input prep (folds gamma/beta/biases)."""
    import ml_dtypes
    bf = ml_dtypes.bfloat16
    wq_e = (wq * gamma[None, :]).astype(np.float32)
    wk_e = (wk * gamma[None, :]).astype(np.float32)
    wv_e = (wv * gamma[None, :]).astype(np.float32)
    bq_e = (bq + wq @ beta).astype(np.float32)
    bk_e = (bk + wk @ beta).astype(np.float32)
    bv_e = (bv + wv @ beta).astype(np.float32)
    bp_e = (bp + wp @ bv_e).astype(np.float32)

    wq_bd = np.zeros((128, 64), np.float32)
    wk_bd = np.zeros((128, 64), np.float32)
    wv_rhs = np.zeros((128, 18), np.float32)
    bq_col = np.zeros((64, 1), np.float32)
    bk_col = np.zeros((64, 1), np.float32)
    fi = np.arange(T)
    ci = np.arange(C)
    for l in range(2):
        n = 2 * r + l
        wq_bd[fi[:, None] * 16 + ci[None, :], (l * 32 + fi)[:, None]] = wq_e[n]
        wk_bd[fi[:, None] * 16 + ci[None, :], (l * 32 + fi)[:, None]] = wk_e[n]
        wv_rhs[fi[:, None] * 16 + ci[None, :],
               (l * 9 + 1 + fi)[:, None]] = wv_e[n]
        bq_col[l * 32 + fi, 0] = bq_e[n]
        bk_col[l * 32 + fi, 0] = bk_e[n]
    # p-conv lhsT rows are in (c,f) order to match the gathered layout
    wp_bd = np.zeros((128, 128), np.float32)
    bp_col = np.zeros((128, 1), np.float32)
    for f in range(T):
        wp_bd[np.ix_(ci * 8 + f, f * 16 + ci)] = wp.T
        bp_col[f * 16 + ci, 0] = bp_e
    selm = np.zeros((128, 16), np.float32)
    selm[np.arange(128), np.tile(ci, T)] = 1.0
    return dict(x=x128, wq_bd=wq_bd.astype(bf), wk_bd=wk_bd.astype(bf),
                wv_rhs=wv_rhs.astype(bf), bq_col=bq_col, bk_col=bk_col,
                wp_bd=wp_bd.astype(bf), bp_col=bp_col, sel=selm)


def make_in_maps(inputs):
    x = np.ascontiguousarray(np.asarray(inputs["x"], np.float32))
    x128 = x.reshape(128, HW)
    args = {k: np.asarray(v, np.float32) for k, v in inputs.items()
            if k != "x"}
    return [host_inputs(r, x128, **args) for r in range(NCORES)]


def run(inputs, trace=False):
    """Returns (out (8,16,64,64) f32, BassKernelResults)."""
    from concourse.bass_utils import run_bass_kernel_spmd
    if "nc" not in _CACHE:
        _CACHE["nc"] = _build_program()
    nc = _CACHE["nc"]
    in_maps = make_in_maps(inputs)
    res = run_bass_kernel_spmd(nc, in_maps, list(range(NCORES)), trace=trace)
    out = np.asarray(res.results[0]["out"], np.float32).reshape(T, C, 64, 64)
    return out, res


def kernel(**inputs):
    out, _ = run(inputs, trace=False)
    return out
